# revision 25
# baseline (speedup 1.0000x reference)
"""Causal self-attention (GQA + RoPE) on 8 Trainium2 NeuronCores.

Sharding: core c = (b, g) with b = c // 4 (batch), g = c % 4 (group of 4
consecutive Q heads; KV head g // 2). Each core computes the attention
output for its 4 heads and a partial out-projection through the matching
256-column slice of Wo. Host sums the 4 partials per batch and adds bo.

v3 design (fp8 DoubleRow heavy):
  - Q projection in fp8-DR (weights host-scaled x64), V+K projection bf16.
  - RoPE in a deinterleaved head-dim layout (host permutes Wq/Wk rows) so
    the rotation is pure DVE mul/add; outputs q8/kz8 written directly fp8.
  - Scores all fp8-DR: stationary kz8 tiles store [k-chunk | zeros] pairs
    so the DR slab-sum contracts 64 real dims at 0.5 cyc/row; the causal
    mask is accumulated into score PSUM by a tiny fp8-DR matmul
    (8*I slab x (-240) mask = -1920).
  - AV: full blocks fp8-DR (vA8/e8), diagonal band bf16 (accuracy for
    early rows). Out-projection bf16 (fp8 there breaks tolerance).
  - Emission interleaves proj(ch+1) and outproj(ch-1) into att(ch) so the
    PE stays busy (p-state) while Scalar streams the exps.
"""

import sys

for _p in ("/opt/trn_rl_repo", "/opt/pypackages"):
    if _p not in sys.path:
        sys.path.append(_p)

from contextlib import ExitStack

import numpy as np

import concourse.bacc as bacc
import concourse.mybir as mybir
import concourse.tile as tile
from concourse.bass import ts
from concourse.bass_utils import run_bass_kernel_spmd

B, T, C = 2, 2048, 1024
HQ, HKV, HD = 16, 2, 64
F32 = mybir.dt.float32
FP8 = mybir.dt.float8e4
DR = mybir.MatmulPerfMode.DoubleRow
BF16 = mybir.dt.bfloat16
AF = mybir.ActivationFunctionType
ALU = mybir.AluOpType
NCC = C // 128        # 8 chunks of the contraction dim
NC16 = T // 128       # 16 k-chunks of 128
SCALE = 1.0 / 64.0    # the reference's double 1/sqrt(64) scaling
WS = 1.0 / 64.0       # fp8 weight descale (weights host-scaled x64)


def _r2(ap):
    """[128, 256] -> [128, 2, 128] slab view for DoubleRow."""
    return ap.rearrange("p (k w) -> p k w", k=2)


def _emit(nc, tc, ctx, d):
    sing = ctx.enter_context(tc.tile_pool(name="sing", bufs=1))

    xT_sb = [sing.tile([128, NCC, 512], BF16, name=f"x{c}") for c in range(4)]
    wkv_sb = sing.tile([128, NCC, 128], BF16)
    wq_sb = sing.tile([128, NCC, 256], BF16)
    bq_sb = sing.tile([128, 2], F32)
    bkv_sb = sing.tile([128, 1], F32)
    wo_sb = sing.tile([128, 2, C], BF16)
    cs_sb = [sing.tile([128, 2, 512], BF16, name=f"cs{c}") for c in range(4)]
    id_sb = sing.tile([64, 64], BF16)
    mi8_sb = sing.tile([128, 2, 128], FP8)
    dmq_sb = sing.tile([128, 2, 128], FP8)

    kvT_sb = sing.tile([128, T], BF16)        # v at 0:64, k (pre-rope, dein) at 64:128
    qT_sb = sing.tile([128, 2, T], BF16)      # biased q pre-rope, dein pair layout
    q8_sb = sing.tile([128, 2, 4, 512], FP8)  # roped q fp8
    kz8a = sing.tile([128, NC16, 256], FP8)   # [k|0] chunks; k at parts 0:64
    kz8b = sing.tile([128, NC16, 256], FP8)   # [k|0] chunks; k at parts 64:128
    vA_sb = sing.tile([128, 16, 128], BF16)   # ones col 0, zeros, v at 64:128
    vA8_sb = sing.tile([128, 16, 128], FP8)   # fp8 copy for DoubleRow AV
    yT_sb = sing.tile([128, 2, T], BF16)      # normalized attention out

    # input DMAs, ordered so the first projection can start early; cos/sin
    # land as 32-partition replication DMAs (4x less input traffic)
    xp = d["xT"].ap()     # [128, 4*NCC*512] (ch, cc) chunks
    cup = d["csu"].ap()   # [32, 3*T] = cos | +sin | -sin
    nc.sync.dma_start(out=id_sb[:], in_=d["ident"].ap())
    nc.sync.dma_start(out=wkv_sb[:], in_=d["wkv"].ap())
    nc.sync.dma_start(out=bkv_sb[:], in_=d["bkv"].ap())
    nc.sync.dma_start(out=wq_sb[:], in_=d["wq"].ap())
    nc.sync.dma_start(out=bq_sb[:], in_=d["bq"].ap())
    for cc in range(NCC):
        nc.sync.dma_start(out=xT_sb[0][:, cc, :], in_=xp[:, ts(cc, 512)])
    for ch in range(4):
        tsl = ts(ch, 512)
        for b32 in range(4):
            nc.sync.dma_start(out=cs_sb[ch][32 * b32:32 * b32 + 32, 0, :],
                              in_=cup[:, tsl])
            sgn = b32 % 2  # [+sin | -sin | +sin | -sin]
            nc.sync.dma_start(out=cs_sb[ch][32 * b32:32 * b32 + 32, 1, :],
                              in_=cup[:, (1 + sgn) * T + 512 * ch:
                                      (1 + sgn) * T + 512 * ch + 512])
    nc.sync.dma_start(out=mi8_sb[:], in_=d["mi8"].ap())
    nc.sync.dma_start(out=dmq_sb[:], in_=d["dmq"].ap())
    for ch in range(1, 4):
        for cc in range(NCC):
            nc.sync.dma_start(out=xT_sb[ch][:, cc, :],
                              in_=xp[:, ch * NCC * 512 + cc * 512:
                                      ch * NCC * 512 + cc * 512 + 512])
        if ch == 1:
            nc.sync.dma_start(out=wo_sb[:], in_=d["wo"].ap())

    # preload the exp act table while the input DMAs stream
    warm = sing.tile([1, 1], F32)
    nc.scalar.activation(out=warm[:], in_=dmq_sb[0:1, 0, 0:1], func=AF.Exp, scale=1.0)


    # one-off zero/one constants (GpSimd so DVE stays free)
    nc.gpsimd.memset(kz8a[64:128, :, 0:128], 0.0)
    nc.gpsimd.memset(kz8a[:, :, 128:256], 0.0)
    nc.gpsimd.memset(kz8b[0:64, :, 0:128], 0.0)
    nc.gpsimd.memset(kz8b[:, :, 128:256], 0.0)
    nc.gpsimd.memset(vA_sb[:, :, 0:1], 1.0)
    nc.gpsimd.memset(vA_sb[:, :, 1:64], 0.0)
    nc.gpsimd.tensor_copy(vA8_sb[:, :, 0:64], vA_sb[:, :, 0:64])

    pw = ctx.enter_context(tc.tile_pool(name="pw", bufs=2, space="PSUM"))
    psp = ctx.enter_context(tc.tile_pool(name="psp", bufs=2, space="PSUM"))
    pyp = ctx.enter_context(tc.tile_pool(name="pyp", bufs=2, space="PSUM"))
    ep = ctx.enter_context(tc.tile_pool(name="ep", bufs=6))
    tmp = ctx.enter_context(tc.tile_pool(name="tmp", bufs=2))
    nrm = ctx.enter_context(tc.tile_pool(name="nrm", bufs=2))
    ost = ctx.enter_context(tc.tile_pool(name="ost", bufs=3))

    def cbc(ap):
        return ap.unsqueeze(1).broadcast_to([ap.shape[0], 2, 512])

    # ---- projection segments for chunk ch ----
    def proj_segs(ch):
        tc_cols = ts(ch, 512)
        assist = ch == 0  # scalar engine handles evacs only while truly idle
        segs = []

        def kv_mm():
            kvp = pw.tile([128, 512], F32, tag="w")
            for cc in range(NCC):
                nc.tensor.matmul(
                    kvp[:], wkv_sb[:, cc, :], xT_sb[ch][:, cc, :],
                    start=(cc == 0), stop=(cc == NCC - 1),
                )
            if assist:
                nc.scalar.activation(out=kvT_sb[:, tc_cols], in_=kvp[:],
                                     func=AF.Identity, bias=bkv_sb[:, 0:1])
            else:
                nc.vector.tensor_scalar_add(kvT_sb[:, tc_cols], kvp[:], bkv_sb[:, 0:1])
        segs.append(kv_mm)

        def k_rope():
            # sm holds [+sin,-sin,+sin,-sin] per 32-block so every operand is
            # read at its in0 base partition (walrus same-base-partition rule)
            cs, sm = cs_sb[ch][:, 0, :], cs_sb[ch][:, 1, :]
            t_k = tmp.tile([64, 512], BF16, tag="tk")
            u_k = tmp.tile([64, 512], BF16, tag="uk")
            nc.vector.tensor_mul(t_k[:], kvT_sb[64:128, tc_cols], cs[64:128, :])
            nc.vector.tensor_mul(u_k[0:32, :], kvT_sb[96:128, tc_cols], sm[96:128, :])
            nc.vector.tensor_mul(u_k[32:64, :], kvT_sb[64:96, tc_cols], sm[64:96, :])
            kz_out = kz8a[0:64, 4 * ch:4 * ch + 4, 0:128]
            nc.vector.tensor_add(kz_out, t_k[:].rearrange("p (a b) -> p a b", a=4),
                                 u_k[:].rearrange("p (a b) -> p a b", a=4))
            nc.scalar.dma_start(out=kz8b[64:128, 4 * ch:4 * ch + 4, 0:128],
                                in_=kz8a[0:64, 4 * ch:4 * ch + 4, 0:128])
        segs.append(k_rope)

        def q_mm(j):
            qp = pw.tile([128, 512], F32, tag="w")
            for cc in range(NCC):
                nc.tensor.matmul(
                    qp[:], wq_sb[:, cc, ts(j, 128)], xT_sb[ch][:, cc, :],
                    start=(cc == 0), stop=(cc == NCC - 1),
                )
            if assist:
                nc.scalar.activation(out=qT_sb[:, j, tc_cols], in_=qp[:],
                                     func=AF.Identity, bias=bq_sb[:, j:j + 1])
            else:
                nc.vector.tensor_scalar_add(qT_sb[:, j, tc_cols], qp[:],
                                            bq_sb[:, j:j + 1])
        segs.append(lambda: q_mm(0))
        segs.append(lambda: q_mm(1))

        def q_rope():
            cs, sm = cs_sb[ch][:, 0, :], cs_sb[ch][:, 1, :]
            qv = qT_sb[:, :, tc_cols]
            t_q = tmp.tile([128, 2, 512], BF16, tag="tq")
            u_q = tmp.tile([128, 2, 512], BF16, tag="uq")
            nc.vector.tensor_mul(t_q[:], qv, cbc(cs))
            # swap even<->odd 32-blocks within each head half; sm is indexed
            # at in0's partitions (sign pattern pre-arranged on host)
            for s0 in (0, 64):
                nc.vector.tensor_mul(u_q[s0:s0 + 32, :, :],
                                     qT_sb[s0 + 32:s0 + 64, :, tc_cols],
                                     cbc(sm[s0 + 32:s0 + 64, :]))
                nc.vector.tensor_mul(u_q[s0 + 32:s0 + 64, :, :],
                                     qT_sb[s0:s0 + 32, :, tc_cols],
                                     cbc(sm[s0:s0 + 32, :]))
            nc.vector.tensor_add(q8_sb[:, :, ch, :], t_q[:], u_q[:])
        segs.append(q_rope)

        def v_trans():
            for r in range(4):
                c16 = 4 * ch + r
                pv = pw.tile([128, 64], BF16, tag="w")
                nc.tensor.transpose(pv[:], kvT_sb[0:64, ts(c16, 128)], id_sb[:])
                nc.vector.tensor_copy(vA_sb[:, c16, 64:128], pv[:])
                nc.gpsimd.tensor_copy(vA8_sb[:, c16, 64:128], vA_sb[:, c16, 64:128])
        segs.append(v_trans)
        return segs

    # ---- attention for one (chunk, head): list of group closures + norm ----
    def att_head(qb, h):
        j = h // 2
        kz = kz8a if h % 2 == 0 else kz8b
        qch = q8_sb[:, j, qb, :]                  # [128, 512] roped q chunk
        # moving slab pair reads the window twice (stride-0); the second
        # slab is nulled by the zero half of the kz tiles
        qrhs = qch.unsqueeze(1).broadcast_to([128, 2, 512])
        box = {}

        def getpy():
            if "py" not in box:
                box["py"] = pyp.tile([128, 512], F32, tag="y", name="py")
            return box["py"]

        groups = []

        def full_pair(p):
            py = getpy()
            st = psp.tile([128, 2, 512], F32, tag="s")
            for i in range(2):
                nc.tensor.matmul(
                    st[:, i, :], _r2(kz[:, 2 * p + i, :]), qrhs,
                    start=True, stop=True, perf_mode=DR,
                )
            e8 = ep.tile([128, 2, 512], FP8, tag="e8")
            nc.scalar.activation(out=e8[:], in_=st[:], func=AF.Exp, scale=SCALE)
            nc.tensor.matmul(
                py[:], vA8_sb[:, 2 * p:2 * p + 2, :], e8[:],
                start=(p == 0), stop=False, perf_mode=DR,
            )
        for p in range(2 * qb):
            groups.append(lambda p=p: full_pair(p))

        def diag(ra):
            py = getpy()
            rb = ra + 1
            wa, wb = 512 - 128 * ra, 512 - 128 * rb
            off_b = 512 if ra == 0 else wa
            st = psp.tile([128, 1024], F32, tag="s")
            nc.tensor.matmul(
                st[:, 0:wa], _r2(kz[:, 4 * qb + ra, :]),
                qch[:, 128 * ra:512].unsqueeze(1).broadcast_to([128, 2, wa]),
                start=True, stop=True, perf_mode=DR, skip_group_check=True,
            )
            nc.tensor.matmul(
                st[:, 0:128], mi8_sb[:], dmq_sb[:],
                start=False, stop=True, perf_mode=DR, skip_group_check=True,
            )
            nc.tensor.matmul(
                st[:, off_b:off_b + wb], _r2(kz[:, 4 * qb + rb, :]),
                qch[:, 128 * rb:512].unsqueeze(1).broadcast_to([128, 2, wb]),
                start=True, stop=True, perf_mode=DR, skip_group_check=True,
            )
            nc.tensor.matmul(
                st[:, off_b:off_b + 128], mi8_sb[:], dmq_sb[:],
                start=False, stop=True, perf_mode=DR, skip_group_check=True,
            )
            we = off_b + wb
            e = ep.tile([128, 1024], BF16, tag="e")
            nc.scalar.activation(out=e[:, 0:we], in_=st[:, 0:we], func=AF.Exp,
                                 scale=SCALE)
            nc.tensor.matmul(
                py[:, 128 * ra:512], vA_sb[:, 4 * qb + ra, :], e[:, 0:wa],
                start=(qb == 0 and ra == 0), stop=False,
            )
            nc.tensor.matmul(
                py[:, 128 * rb:512], vA_sb[:, 4 * qb + rb, :], e[:, off_b:off_b + wb],
                start=False, stop=(rb == 3),
            )
        groups.append(lambda: diag(0))
        groups.append(lambda: diag(2))

        def normalize():
            # 1/den (DVE), broadcast across partitions (GpSimd), then scale
            # while evacuating the AV PSUM (DVE). Emitted one head late so
            # the in-order DVE/GpSimd queues never stall the live head.
            py = box["py"]
            rdr = nrm.tile([1, 512], F32, tag="rdr")
            nc.vector.reciprocal_approx_fast(rdr[:], py[0:1, :])
            pbs = nrm.tile([64, 512], F32, tag="pbs")
            nc.gpsimd.partition_broadcast(pbs[:], rdr[:])
            b0 = (h % 2) * 64
            nc.vector.tensor_mul(
                yT_sb[b0:b0 + 64, j, ts(qb, 512)], py[64:128, :], pbs[:],
            )
        return groups, normalize

    # ---- out-projection unit (one tq of 128 t-rows, one 512-col half) ----
    def outproj_u(tq, cf, scalar_evac=False):
        po = pw.tile([128, 512], F32, tag="w")
        for j in range(2):
            nc.tensor.matmul(
                po[:], yT_sb[:, j, ts(tq, 128)], wo_sb[:, j, ts(cf, 512)],
                start=(j == 0), stop=(j == 1),
            )
        ob = ost.tile([128, 512], BF16, tag="ob")
        if scalar_evac:
            nc.scalar.activation(out=ob[:], in_=po[:], func=AF.Identity)
        else:
            nc.vector.tensor_copy(ob[:], po[:])
        nc.sync.dma_start(out=d["out"].ap()[ts(tq, 128), ts(cf, 512)], in_=ob[:])

    # ---- emission schedule. Fillers are placed by dependency-readiness so
    # they never block the in-order engine queues: the (ch+1) projection
    # matmuls (only need x) go right after h0's groups; the rope (DVE-only)
    # after the evacs have had time; v_trans (PE transpose needing the kv
    # evac) last; outproj units (all deps a chunk old) spread throughout.
    s0 = proj_segs(0)
    for i in (0, 2, 3, 1, 4, 5):
        s0[i]()
    pending_norm = None
    for ch in range(4):
        segs = proj_segs(ch + 1) if ch < 3 else []
        # emission order preserved: [kv_mm, q_mm0, q_mm1, k_rope, q_rope,
        # v_trans] -- mm-only segs land early (deps: just x), rope after its
        # evacs, v_trans last (the kv evac is long-retired by then)
        segq = [segs[0], segs[2], segs[3], segs[1], segs[4], segs[5]] if segs else []
        seg_plan = {0: 3, 1: 2, 3: 1}
        ops = []
        if ch >= 1:
            for tq in range(4 * (ch - 1), 4 * ch):
                for cf in range(2):
                    ops.append(lambda tq=tq, cf=cf: outproj_u(tq, cf))
        npoints = 4 * (2 * ch + 2)
        pace = max(1, npoints // max(1, len(ops)))
        oi = 0
        pt = 0
        carry = 0
        for h in range(4):
            budget = seg_plan.get(h, 0) + carry
            groups, norm = att_head(ch, h)
            for gi, g in enumerate(groups):
                g()
                if gi == 0 and pending_norm is not None:
                    pending_norm()
                    pending_norm = None
                if budget > 0 and segq:
                    segq.pop(0)()
                    budget -= 1
                pt += 1
                if pt % pace == 0 and oi < len(ops):
                    ops[oi]()
                    oi += 1
            carry = budget
            pending_norm = norm
        for seg in segq:
            seg()
        while oi < len(ops):
            ops[oi]()
            oi += 1
    pending_norm()
    for tq in range(12, 16):
        for cf in range(2):
            outproj_u(tq, cf, scalar_evac=(cf == 1))


def build_program(num_devices=8):
    nc = bacc.Bacc("TRN2", target_bir_lowering=False, debug=False,
                   num_devices=num_devices)
    d = {}
    spec = [
        ("xT", [128, 4 * NCC * 512], BF16),
        ("wkv", [128, NCC * 128], BF16),
        ("wq", [128, NCC * 256], BF16),
        ("bq", [128, 2], F32),
        ("bkv", [128, 1], F32),
        ("wo", [128, 2 * C], BF16),
        ("csu", [32, 3 * T], BF16),
        ("ident", [64, 64], BF16),
        ("mi8", [128, 2 * 128], FP8),
        ("dmq", [128, 2 * 128], FP8),
    ]
    for name, shape, dt in spec:
        d[name] = nc.dram_tensor(name, shape, dt, kind="ExternalInput")
    d["out"] = nc.dram_tensor("out", [T, C], BF16, kind="ExternalOutput")
    with tile.TileContext(nc) as tc, ExitStack() as ctx:
        _emit(nc, tc, ctx, d)
    nc.compile()
    return nc


def host_prep(inputs):
    """Slice/permute the full inputs into the 8 per-core input maps."""
    import ml_dtypes
    E4 = ml_dtypes.float8_e4m3
    bf = lambda a: np.ascontiguousarray(a.astype(ml_dtypes.bfloat16))
    f8c = lambda a: np.ascontiguousarray(a.astype(np.float32).astype(E4))
    f = lambda a: np.ascontiguousarray(np.asarray(a, dtype=np.float32))
    x, rc = f(inputs["x"]), f(inputs["rope_cache"])
    Wq, bq = f(inputs["Wq"]), f(inputs["bq"])
    Wk, bk = f(inputs["Wk"]), f(inputs["bk"])
    Wv, bv = f(inputs["Wv"]), f(inputs["bv"])
    Wo = f(inputs["Wo"])

    cos, sin = rc[:, 1::2], rc[:, 0::2]          # [T, 32]
    # dein partition layout: [h-even evens | h-even odds | h-odd evens | h-odd odds]
    po = np.arange(128)
    parity = po // 64                             # head within pair
    dd = 2 * (po % 32) + (po // 32) % 2           # orig hd dim
    ko = np.arange(64)
    kd = 2 * (ko % 32) + (ko // 32)               # k dein dim order (evens|odds)

    # unique cos/sin rows; on-chip the [128,T] tiles are built by
    # replication DMAs with the [+ - + -] sin sign pattern per 32-block
    csu = np.concatenate([cos.T, sin.T, -sin.T], axis=1)  # [32, 3*T]

    ident = np.eye(64, dtype=np.float32)
    kk, qq = np.arange(128)[:, None], np.arange(128)[None, :]
    mi8 = np.zeros((128, 2, 128), np.float32)
    mi8[:, 0, :] = 8.0 * np.eye(128)
    dmq = np.zeros((128, 2, 128), np.float32)
    dmq[:, 0, :] = np.where(kk > qq, -240.0, 0.0)

    xsw = lambda a: a.T.reshape(NCC, 128, 4, 512).transpose(1, 2, 0, 3).reshape(128, -1)

    in_maps = []
    for core in range(8):
        b, g = core // 4, core % 4
        kv = g // 2
        # wkv: [Wv unperm | Wk dein] per cc chunk
        wv = Wv[64 * kv:64 * (kv + 1)].T          # [C, 64]
        wk = Wk[64 * kv:64 * (kv + 1)][kd].T      # [C, 64] dein row order
        wkv = np.concatenate([wv, wk], axis=1)    # [C, 128]
        wkv = wkv.reshape(NCC, 128, 128).transpose(1, 0, 2).reshape(128, -1)
        # wq: dein-permuted columns, [C, (cc), j*128+po]
        wq_rows = np.empty((2, 128), np.int64)
        for j in range(2):
            wq_rows[j] = 256 * g + 64 * (2 * j + parity) + dd
        wqt = Wq[wq_rows.reshape(-1)].T.reshape(C, 2, 128)  # [C, j, po]
        wqp = wqt.reshape(NCC, 128, 256).transpose(1, 0, 2).reshape(128, -1)
        bq_p = np.stack([bq[wq_rows[0]], bq[wq_rows[1]]], axis=1)    # [128, 2]
        bkv_p = np.concatenate([bv[64 * kv:64 * (kv + 1)],
                                bk[64 * kv:64 * (kv + 1)][kd]]).reshape(128, 1)
        in_maps.append({
            "xT": bf(xsw(x[b])),
            "wkv": bf(wkv),
            "wq": bf(wqp),
            "bq": np.ascontiguousarray(bq_p),
            "bkv": np.ascontiguousarray(bkv_p),
            "wo": bf(Wo[:, 256 * g:256 * (g + 1)].T.reshape(2, 128, C)
                     .transpose(1, 0, 2).reshape(128, -1)),
            "csu": bf(csu),
            "ident": bf(ident),
            "mi8": np.ascontiguousarray(mi8.reshape(128, -1).astype(E4)),
            "dmq": np.ascontiguousarray(dmq.reshape(128, -1).astype(E4)),
        })
    return in_maps


_PROGRAM = None


def _get_program():
    global _PROGRAM
    if _PROGRAM is None:
        _PROGRAM = build_program()
    return _PROGRAM


def _gather(results, bo):
    full = np.empty((B, T, C), np.float32)
    for b in range(B):
        acc = results[4 * b]["out"].astype(np.float32).copy()
        for g in range(1, 4):
            acc += results[4 * b + g]["out"]
        full[b] = acc + bo
    return full


def kernel(**inputs):
    nc = _get_program()
    in_maps = host_prep(inputs)
    res = run_bass_kernel_spmd(nc, in_maps, list(range(8)))
    return _gather(res.results, np.asarray(inputs["bo"], np.float32))


def kernel_traced(**inputs):
    """Like kernel() but with NTFF tracing; returns (output, BassKernelResults)."""
    nc = _get_program()
    in_maps = host_prep(inputs)
    res = run_bass_kernel_spmd(nc, in_maps, list(range(8)), trace=True)
    return _gather(res.results, np.asarray(inputs["bo"], np.float32)), res


# revision 26
# speedup vs baseline: 1.0427x; 1.0427x over previous
"""Causal self-attention (GQA + RoPE) on 8 Trainium2 NeuronCores.

Sharding: core c = (b, g) with b = c // 4 (batch), g = c % 4 (group of 4
consecutive Q heads; KV head g // 2). Each core computes the attention
output for its 4 heads and a partial out-projection through the matching
256-column slice of Wo. Host sums the 4 partials per batch and adds bo.

v3 design (fp8 DoubleRow heavy):
  - Q projection in fp8-DR (weights host-scaled x64), V+K projection bf16.
  - RoPE in a deinterleaved head-dim layout (host permutes Wq/Wk rows) so
    the rotation is pure DVE mul/add; outputs q8/kz8 written directly fp8.
  - Scores all fp8-DR: stationary kz8 tiles store [k-chunk | zeros] pairs
    so the DR slab-sum contracts 64 real dims at 0.5 cyc/row; the causal
    mask is accumulated into score PSUM by a tiny fp8-DR matmul
    (8*I slab x (-240) mask = -1920).
  - AV: full blocks fp8-DR (vA8/e8), diagonal band bf16 (accuracy for
    early rows). Out-projection bf16 (fp8 there breaks tolerance).
  - Emission interleaves proj(ch+1) and outproj(ch-1) into att(ch) so the
    PE stays busy (p-state) while Scalar streams the exps.
"""

import sys

for _p in ("/opt/trn_rl_repo", "/opt/pypackages"):
    if _p not in sys.path:
        sys.path.append(_p)

from contextlib import ExitStack

import numpy as np

import concourse.bacc as bacc
import concourse.mybir as mybir
import concourse.tile as tile
from concourse.bass import ts
from concourse.bass_utils import run_bass_kernel_spmd

B, T, C = 2, 2048, 1024
HQ, HKV, HD = 16, 2, 64
F32 = mybir.dt.float32
FP8 = mybir.dt.float8e4
DR = mybir.MatmulPerfMode.DoubleRow
BF16 = mybir.dt.bfloat16
AF = mybir.ActivationFunctionType
ALU = mybir.AluOpType
NCC = C // 128        # 8 chunks of the contraction dim
NC16 = T // 128       # 16 k-chunks of 128
SCALE = 1.0 / 64.0    # the reference's double 1/sqrt(64) scaling
WS = 1.0 / 64.0       # fp8 weight descale (weights host-scaled x64)


def _r2(ap):
    """[128, 256] -> [128, 2, 128] slab view for DoubleRow."""
    return ap.rearrange("p (k w) -> p k w", k=2)


def _emit(nc, tc, ctx, d):
    sing = ctx.enter_context(tc.tile_pool(name="sing", bufs=1))

    xT_sb = [sing.tile([128, NCC, 512], BF16, name=f"x{c}") for c in range(4)]
    wkv_sb = sing.tile([128, NCC, 128], BF16)
    wq_sb = sing.tile([128, NCC, 256], BF16)
    bq_sb = sing.tile([128, 2], F32)
    bkv_sb = sing.tile([128, 1], F32)
    wo_sb = sing.tile([128, 2, C], BF16)
    cs_sb = [sing.tile([128, 2, 512], BF16, name=f"cs{c}") for c in range(4)]
    id_sb = sing.tile([64, 64], BF16)
    mi8_sb = sing.tile([128, 2, 128], FP8)
    dmq_sb = sing.tile([128, 2, 128], FP8)

    kvT_sb = sing.tile([128, T], BF16)        # v at 0:64, k (pre-rope, dein) at 64:128
    qT_sb = sing.tile([128, 2, T], BF16)      # biased q pre-rope, dein pair layout
    q8_sb = sing.tile([128, 2, 4, 512], FP8)  # roped q fp8
    kz8a = sing.tile([128, NC16, 256], FP8)   # [k|0] chunks; k at parts 0:64
    kz8b = sing.tile([128, NC16, 256], FP8)   # [k|0] chunks; k at parts 64:128
    vA_sb = sing.tile([128, 16, 128], BF16)   # ones col 0, zeros, v at 64:128
    vA8_sb = sing.tile([128, 16, 128], FP8)   # fp8 copy for DoubleRow AV
    yT_sb = sing.tile([128, 2, T], BF16)      # normalized attention out

    # input DMAs: few, large instructions (the sync sequencer issues DMAs
    # serially at ~0.5us each, so instruction count is the scarce resource).
    # csu is a 64-row [cos;cos | +sin;-sin] pattern: each [128,...] cos/sin
    # tile is two 64-partition replication DMAs.
    xp = d["xT"].ap()     # [128, 4*NCC*512] (ch, cc) chunks
    cup = d["csu"].ap().rearrange("p (k t) -> p k t", k=2)   # [64, 2, T]
    nc.sync.dma_start(out=id_sb[:], in_=d["ident"].ap())
    nc.sync.dma_start(out=wkv_sb[:], in_=d["wkv"].ap())
    nc.sync.dma_start(out=bkv_sb[:], in_=d["bkv"].ap())
    nc.sync.dma_start(out=wq_sb[:], in_=d["wq"].ap())
    nc.sync.dma_start(out=bq_sb[:], in_=d["bq"].ap())
    nc.sync.dma_start(out=xT_sb[0][:, 0:4, :], in_=xp[:, 0:2048])
    nc.sync.dma_start(out=xT_sb[0][:, 4:8, :], in_=xp[:, 2048:4096])
    nc.sync.dma_start(out=mi8_sb[:], in_=d["mi8"].ap())
    nc.sync.dma_start(out=dmq_sb[:], in_=d["dmq"].ap())
    for ch in range(4):
        nc.sync.dma_start(out=cs_sb[ch][0:64, :, :], in_=cup[:, :, ts(ch, 512)])
        nc.sync.dma_start(out=cs_sb[ch][64:128, :, :], in_=cup[:, :, ts(ch, 512)])
        if ch >= 1:
            nc.sync.dma_start(out=xT_sb[ch][:], in_=xp[:, ts(ch, NCC * 512)])
        if ch == 1:
            nc.sync.dma_start(out=wo_sb[:], in_=d["wo"].ap())

    # preload the exp act table while the input DMAs stream
    warm = sing.tile([1, 1], F32)
    nc.scalar.activation(out=warm[:], in_=id_sb[0:1, 0:1], func=AF.Exp, scale=1.0)


    # one-off zero/one constants (GpSimd so DVE stays free)
    nc.gpsimd.memset(kz8a[64:128, :, 0:128], 0.0)
    nc.gpsimd.memset(kz8a[:, :, 128:256], 0.0)
    nc.gpsimd.memset(kz8b[0:64, :, 0:128], 0.0)
    nc.gpsimd.memset(kz8b[:, :, 128:256], 0.0)
    nc.gpsimd.memset(vA_sb[:, :, 0:1], 1.0)
    nc.gpsimd.memset(vA_sb[:, :, 1:64], 0.0)
    nc.gpsimd.tensor_copy(vA8_sb[:, :, 0:64], vA_sb[:, :, 0:64])

    pw = ctx.enter_context(tc.tile_pool(name="pw", bufs=2, space="PSUM"))
    psp = ctx.enter_context(tc.tile_pool(name="psp", bufs=2, space="PSUM"))
    pyp = ctx.enter_context(tc.tile_pool(name="pyp", bufs=2, space="PSUM"))
    ep = ctx.enter_context(tc.tile_pool(name="ep", bufs=6))
    tmp = ctx.enter_context(tc.tile_pool(name="tmp", bufs=2))
    nrm = ctx.enter_context(tc.tile_pool(name="nrm", bufs=2))
    ost = ctx.enter_context(tc.tile_pool(name="ost", bufs=3))

    def cbc(ap):
        return ap.unsqueeze(1).broadcast_to([ap.shape[0], 2, 512])

    # ---- projection segments for chunk ch ----
    def proj_segs(ch):
        tc_cols = ts(ch, 512)
        assist = ch == 0  # scalar engine handles evacs only while truly idle
        segs = []

        def kv_mm():
            kvp = pw.tile([128, 512], F32, tag="w")
            for cc in range(NCC):
                nc.tensor.matmul(
                    kvp[:], wkv_sb[:, cc, :], xT_sb[ch][:, cc, :],
                    start=(cc == 0), stop=(cc == NCC - 1),
                )
            if assist:
                nc.scalar.activation(out=kvT_sb[:, tc_cols], in_=kvp[:],
                                     func=AF.Identity, bias=bkv_sb[:, 0:1])
            else:
                nc.vector.tensor_scalar_add(kvT_sb[:, tc_cols], kvp[:], bkv_sb[:, 0:1])
        segs.append(kv_mm)

        def k_rope():
            # sm holds [+sin,-sin,+sin,-sin] per 32-block so every operand is
            # read at its in0 base partition (walrus same-base-partition rule)
            cs, sm = cs_sb[ch][:, 0, :], cs_sb[ch][:, 1, :]
            t_k = tmp.tile([64, 512], BF16, tag="tk")
            u_k = tmp.tile([64, 512], BF16, tag="uk")
            nc.vector.tensor_mul(t_k[:], kvT_sb[64:128, tc_cols], cs[64:128, :])
            nc.vector.tensor_mul(u_k[0:32, :], kvT_sb[96:128, tc_cols], sm[96:128, :])
            nc.vector.tensor_mul(u_k[32:64, :], kvT_sb[64:96, tc_cols], sm[64:96, :])
            kz_out = kz8a[0:64, 4 * ch:4 * ch + 4, 0:128]
            nc.vector.tensor_add(kz_out, t_k[:].rearrange("p (a b) -> p a b", a=4),
                                 u_k[:].rearrange("p (a b) -> p a b", a=4))
            nc.scalar.dma_start(out=kz8b[64:128, 4 * ch:4 * ch + 4, 0:128],
                                in_=kz8a[0:64, 4 * ch:4 * ch + 4, 0:128])
        segs.append(k_rope)

        def q_mm(j):
            qp = pw.tile([128, 512], F32, tag="w")
            for cc in range(NCC):
                nc.tensor.matmul(
                    qp[:], wq_sb[:, cc, ts(j, 128)], xT_sb[ch][:, cc, :],
                    start=(cc == 0), stop=(cc == NCC - 1),
                )
            if assist:
                nc.scalar.activation(out=qT_sb[:, j, tc_cols], in_=qp[:],
                                     func=AF.Identity, bias=bq_sb[:, j:j + 1])
            else:
                nc.vector.tensor_scalar_add(qT_sb[:, j, tc_cols], qp[:],
                                            bq_sb[:, j:j + 1])
        segs.append(lambda: q_mm(0))
        segs.append(lambda: q_mm(1))

        def q_rope():
            cs, sm = cs_sb[ch][:, 0, :], cs_sb[ch][:, 1, :]
            qv = qT_sb[:, :, tc_cols]
            t_q = tmp.tile([128, 2, 512], BF16, tag="tq")
            u_q = tmp.tile([128, 2, 512], BF16, tag="uq")
            nc.vector.tensor_mul(t_q[:], qv, cbc(cs))
            # swap even<->odd 32-blocks within each head half; sm is indexed
            # at in0's partitions (sign pattern pre-arranged on host)
            for s0 in (0, 64):
                nc.vector.tensor_mul(u_q[s0:s0 + 32, :, :],
                                     qT_sb[s0 + 32:s0 + 64, :, tc_cols],
                                     cbc(sm[s0 + 32:s0 + 64, :]))
                nc.vector.tensor_mul(u_q[s0 + 32:s0 + 64, :, :],
                                     qT_sb[s0:s0 + 32, :, tc_cols],
                                     cbc(sm[s0:s0 + 32, :]))
            nc.vector.tensor_add(q8_sb[:, :, ch, :], t_q[:], u_q[:])
        segs.append(q_rope)

        def v_trans():
            for r in range(4):
                c16 = 4 * ch + r
                pv = pw.tile([128, 64], BF16, tag="w")
                nc.tensor.transpose(pv[:], kvT_sb[0:64, ts(c16, 128)], id_sb[:])
                nc.vector.tensor_copy(vA_sb[:, c16, 64:128], pv[:])
                nc.gpsimd.tensor_copy(vA8_sb[:, c16, 64:128], vA_sb[:, c16, 64:128])
        segs.append(v_trans)
        return segs

    # ---- attention for one (chunk, head): list of group closures + norm ----
    def att_head(qb, h):
        j = h // 2
        kz = kz8a if h % 2 == 0 else kz8b
        qch = q8_sb[:, j, qb, :]                  # [128, 512] roped q chunk
        # moving slab pair reads the window twice (stride-0); the second
        # slab is nulled by the zero half of the kz tiles
        qrhs = qch.unsqueeze(1).broadcast_to([128, 2, 512])
        box = {}

        def getpy():
            if "py" not in box:
                box["py"] = pyp.tile([128, 512], F32, tag="y", name="py")
            return box["py"]

        groups = []

        def full_pair(p):
            py = getpy()
            st = psp.tile([128, 2, 512], F32, tag="s")
            for i in range(2):
                nc.tensor.matmul(
                    st[:, i, :], _r2(kz[:, 2 * p + i, :]), qrhs,
                    start=True, stop=True, perf_mode=DR,
                )
            e8 = ep.tile([128, 2, 512], FP8, tag="e8")
            nc.scalar.activation(out=e8[:], in_=st[:], func=AF.Exp, scale=SCALE)
            nc.tensor.matmul(
                py[:], vA8_sb[:, 2 * p:2 * p + 2, :], e8[:],
                start=(p == 0), stop=False, perf_mode=DR,
            )
        for p in range(2 * qb):
            groups.append(lambda p=p: full_pair(p))

        def diag(ra):
            py = getpy()
            rb = ra + 1
            wa, wb = 512 - 128 * ra, 512 - 128 * rb
            off_b = 512 if ra == 0 else wa
            st = psp.tile([128, 1024], F32, tag="s")
            nc.tensor.matmul(
                st[:, 0:wa], _r2(kz[:, 4 * qb + ra, :]),
                qch[:, 128 * ra:512].unsqueeze(1).broadcast_to([128, 2, wa]),
                start=True, stop=True, perf_mode=DR, skip_group_check=True,
            )
            nc.tensor.matmul(
                st[:, 0:128], mi8_sb[:], dmq_sb[:],
                start=False, stop=True, perf_mode=DR, skip_group_check=True,
            )
            nc.tensor.matmul(
                st[:, off_b:off_b + wb], _r2(kz[:, 4 * qb + rb, :]),
                qch[:, 128 * rb:512].unsqueeze(1).broadcast_to([128, 2, wb]),
                start=True, stop=True, perf_mode=DR, skip_group_check=True,
            )
            nc.tensor.matmul(
                st[:, off_b:off_b + 128], mi8_sb[:], dmq_sb[:],
                start=False, stop=True, perf_mode=DR, skip_group_check=True,
            )
            we = off_b + wb
            e = ep.tile([128, 1024], BF16, tag="e")
            nc.scalar.activation(out=e[:, 0:we], in_=st[:, 0:we], func=AF.Exp,
                                 scale=SCALE)
            nc.tensor.matmul(
                py[:, 128 * ra:512], vA_sb[:, 4 * qb + ra, :], e[:, 0:wa],
                start=(qb == 0 and ra == 0), stop=False,
            )
            nc.tensor.matmul(
                py[:, 128 * rb:512], vA_sb[:, 4 * qb + rb, :], e[:, off_b:off_b + wb],
                start=False, stop=(rb == 3),
            )
        groups.append(lambda: diag(0))
        groups.append(lambda: diag(2))

        def normalize():
            # 1/den (DVE), broadcast across partitions (GpSimd), then scale
            # while evacuating the AV PSUM (DVE). Emitted one head late so
            # the in-order DVE/GpSimd queues never stall the live head.
            py = box["py"]
            rdr = nrm.tile([1, 512], F32, tag="rdr")
            nc.vector.reciprocal_approx_fast(rdr[:], py[0:1, :])
            pbs = nrm.tile([64, 512], F32, tag="pbs")
            nc.gpsimd.partition_broadcast(pbs[:], rdr[:])
            b0 = (h % 2) * 64
            nc.vector.tensor_mul(
                yT_sb[b0:b0 + 64, j, ts(qb, 512)], py[64:128, :], pbs[:],
            )
        return groups, normalize

    # ---- out-projection unit (one tq of 128 t-rows, one 512-col half) ----
    def outproj_u(tq, cf, scalar_evac=False):
        po = pw.tile([128, 512], F32, tag="w")
        for j in range(2):
            nc.tensor.matmul(
                po[:], yT_sb[:, j, ts(tq, 128)], wo_sb[:, j, ts(cf, 512)],
                start=(j == 0), stop=(j == 1),
            )
        ob = ost.tile([128, 512], BF16, tag="ob")
        if scalar_evac:
            nc.scalar.activation(out=ob[:], in_=po[:], func=AF.Identity)
        else:
            nc.vector.tensor_copy(ob[:], po[:])
        nc.sync.dma_start(out=d["out"].ap()[ts(tq, 128), ts(cf, 512)], in_=ob[:])

    # ---- emission schedule. Fillers are placed by dependency-readiness so
    # they never block the in-order engine queues: the (ch+1) projection
    # matmuls (only need x) go right after h0's groups; the rope (DVE-only)
    # after the evacs have had time; v_trans (PE transpose needing the kv
    # evac) last; outproj units (all deps a chunk old) spread throughout.
    s0 = proj_segs(0)
    for i in (0, 2, 3, 1, 4, 5):
        s0[i]()
    pending_norm = None
    for ch in range(4):
        segs = proj_segs(ch + 1) if ch < 3 else []
        # emission order preserved: [kv_mm, q_mm0, q_mm1, k_rope, q_rope,
        # v_trans] -- mm-only segs land early (deps: just x), rope after its
        # evacs, v_trans last (the kv evac is long-retired by then)
        segq = [segs[0], segs[2], segs[3], segs[1], segs[4], segs[5]] if segs else []
        seg_plan = {0: 3, 1: 2, 3: 1}
        ops = []
        if ch >= 1:
            for tq in range(4 * (ch - 1), 4 * ch):
                for cf in range(2):
                    ops.append(lambda tq=tq, cf=cf: outproj_u(tq, cf))
        npoints = 4 * (2 * ch + 2)
        pace = max(1, npoints // max(1, len(ops)))
        oi = 0
        pt = 0
        carry = 0
        for h in range(4):
            budget = seg_plan.get(h, 0) + carry
            groups, norm = att_head(ch, h)
            for gi, g in enumerate(groups):
                g()
                if gi == 0 and pending_norm is not None:
                    pending_norm()
                    pending_norm = None
                if budget > 0 and segq:
                    segq.pop(0)()
                    budget -= 1
                pt += 1
                if pt % pace == 0 and oi < len(ops):
                    ops[oi]()
                    oi += 1
            carry = budget
            pending_norm = norm
        for seg in segq:
            seg()
        while oi < len(ops):
            ops[oi]()
            oi += 1
    pending_norm()
    for tq in range(12, 16):
        for cf in range(2):
            outproj_u(tq, cf, scalar_evac=(cf == 1))


def build_program(num_devices=8):
    nc = bacc.Bacc("TRN2", target_bir_lowering=False, debug=False,
                   num_devices=num_devices)
    d = {}
    spec = [
        ("xT", [128, 4 * NCC * 512], BF16),
        ("wkv", [128, NCC * 128], BF16),
        ("wq", [128, NCC * 256], BF16),
        ("bq", [128, 2], F32),
        ("bkv", [128, 1], F32),
        ("wo", [128, 2 * C], BF16),
        ("csu", [64, 2 * T], BF16),
        ("ident", [64, 64], BF16),
        ("mi8", [128, 2 * 128], FP8),
        ("dmq", [128, 2 * 128], FP8),
    ]
    for name, shape, dt in spec:
        d[name] = nc.dram_tensor(name, shape, dt, kind="ExternalInput")
    d["out"] = nc.dram_tensor("out", [T, C], BF16, kind="ExternalOutput")
    with tile.TileContext(nc) as tc, ExitStack() as ctx:
        _emit(nc, tc, ctx, d)
    nc.compile()
    return nc


def host_prep(inputs):
    """Slice/permute the full inputs into the 8 per-core input maps."""
    import ml_dtypes
    E4 = ml_dtypes.float8_e4m3
    bf = lambda a: np.ascontiguousarray(a.astype(ml_dtypes.bfloat16))
    f8c = lambda a: np.ascontiguousarray(a.astype(np.float32).astype(E4))
    f = lambda a: np.ascontiguousarray(np.asarray(a, dtype=np.float32))
    x, rc = f(inputs["x"]), f(inputs["rope_cache"])
    Wq, bq = f(inputs["Wq"]), f(inputs["bq"])
    Wk, bk = f(inputs["Wk"]), f(inputs["bk"])
    Wv, bv = f(inputs["Wv"]), f(inputs["bv"])
    Wo = f(inputs["Wo"])

    cos, sin = rc[:, 1::2], rc[:, 0::2]          # [T, 32]
    # dein partition layout: [h-even evens | h-even odds | h-odd evens | h-odd odds]
    po = np.arange(128)
    parity = po // 64                             # head within pair
    dd = 2 * (po % 32) + (po // 32) % 2           # orig hd dim
    ko = np.arange(64)
    kd = 2 * (ko % 32) + (ko // 32)               # k dein dim order (evens|odds)

    # 64-row cos/sin pattern: [cos;cos] and [+sin;-sin]; the [128,T] tiles
    # on chip are two 64-partition replication DMAs each
    cos64 = np.concatenate([cos.T, cos.T], axis=0)          # [64, T]
    sin64 = np.concatenate([sin.T, -sin.T], axis=0)         # [64, T]
    csu = np.stack([cos64, sin64], axis=1).reshape(64, -1)  # [64, 2*T]

    ident = np.eye(64, dtype=np.float32)
    kk, qq = np.arange(128)[:, None], np.arange(128)[None, :]
    mi8 = np.zeros((128, 2, 128), np.float32)
    mi8[:, 0, :] = 8.0 * np.eye(128)
    dmq = np.zeros((128, 2, 128), np.float32)
    dmq[:, 0, :] = np.where(kk > qq, -240.0, 0.0)

    xsw = lambda a: a.T.reshape(NCC, 128, 4, 512).transpose(1, 2, 0, 3).reshape(128, -1)

    in_maps = []
    for core in range(8):
        b, g = core // 4, core % 4
        kv = g // 2
        # wkv: [Wv unperm | Wk dein] per cc chunk
        wv = Wv[64 * kv:64 * (kv + 1)].T          # [C, 64]
        wk = Wk[64 * kv:64 * (kv + 1)][kd].T      # [C, 64] dein row order
        wkv = np.concatenate([wv, wk], axis=1)    # [C, 128]
        wkv = wkv.reshape(NCC, 128, 128).transpose(1, 0, 2).reshape(128, -1)
        # wq: dein-permuted columns, [C, (cc), j*128+po]
        wq_rows = np.empty((2, 128), np.int64)
        for j in range(2):
            wq_rows[j] = 256 * g + 64 * (2 * j + parity) + dd
        wqt = Wq[wq_rows.reshape(-1)].T.reshape(C, 2, 128)  # [C, j, po]
        wqp = wqt.reshape(NCC, 128, 256).transpose(1, 0, 2).reshape(128, -1)
        bq_p = np.stack([bq[wq_rows[0]], bq[wq_rows[1]]], axis=1)    # [128, 2]
        bkv_p = np.concatenate([bv[64 * kv:64 * (kv + 1)],
                                bk[64 * kv:64 * (kv + 1)][kd]]).reshape(128, 1)
        in_maps.append({
            "xT": bf(xsw(x[b])),
            "wkv": bf(wkv),
            "wq": bf(wqp),
            "bq": np.ascontiguousarray(bq_p),
            "bkv": np.ascontiguousarray(bkv_p),
            "wo": bf(Wo[:, 256 * g:256 * (g + 1)].T.reshape(2, 128, C)
                     .transpose(1, 0, 2).reshape(128, -1)),
            "csu": bf(csu),
            "ident": bf(ident),
            "mi8": np.ascontiguousarray(mi8.reshape(128, -1).astype(E4)),
            "dmq": np.ascontiguousarray(dmq.reshape(128, -1).astype(E4)),
        })
    return in_maps


_PROGRAM = None


def _get_program():
    global _PROGRAM
    if _PROGRAM is None:
        _PROGRAM = build_program()
    return _PROGRAM


def _gather(results, bo):
    full = np.empty((B, T, C), np.float32)
    for b in range(B):
        acc = results[4 * b]["out"].astype(np.float32).copy()
        for g in range(1, 4):
            acc += results[4 * b + g]["out"]
        full[b] = acc + bo
    return full


def kernel(**inputs):
    nc = _get_program()
    in_maps = host_prep(inputs)
    res = run_bass_kernel_spmd(nc, in_maps, list(range(8)))
    return _gather(res.results, np.asarray(inputs["bo"], np.float32))


def kernel_traced(**inputs):
    """Like kernel() but with NTFF tracing; returns (output, BassKernelResults)."""
    nc = _get_program()
    in_maps = host_prep(inputs)
    res = run_bass_kernel_spmd(nc, in_maps, list(range(8)), trace=True)
    return _gather(res.results, np.asarray(inputs["bo"], np.float32)), res
